# revision 23
# baseline (speedup 1.0000x reference)
"""Trainium2 Bass kernel for nn_BimodalAttention.

Reference computation (B=128, L=512, D=256, T=64, G=8):
  aco_p/vis_p = group-mean pool (8->1) along L            [B,T,D]
  c_att = sigmoid(cw0*aco_p + cw1*vis_p + cb)             [B,T,D]
  hw    = 0.5*(aco_p+vis_p)
  h_att = sigmoid(hw.mean(D) @ Wh.T + bh)                 [B,T]
  w_att = sigmoid(hw.mean(T) @ Ww.T + bw)                 [B,D]
  scale = (h_att[map] + w_att + c_att[map]) / 3           [B,L,D]
  out   = where(IS_BAG==1, in*scale, in)  for both modalities

Sharding: pure data parallel, 16 batches per core on 8 cores.

Per-core on-chip layout: batch slice [512,256] viewed as [128p, (n=4, d=256)]
with l = n*128 + p, so pooled frame t = 16n + p//8.  Pooling runs on the PE
in bf16 (exact 1/8 selector weights); the pooled->full broadcast runs in
float32r with the w_att row folded in as a 65th contraction row; the /3 and
the IS_BAG mask are applied exactly in fp32 via a per-partition
scale/bias pass (scale = mask/3, bias = 1-mask).
"""

import sys
from contextlib import ExitStack

import numpy as np

sys.path.insert(0, "/opt/trn_rl_repo")

import concourse.bass as bass  # noqa: E402
import concourse.tile as tile  # noqa: E402
from concourse import bacc, mybir  # noqa: E402
from concourse.bass_utils import run_bass_kernel_spmd  # noqa: E402

B, L, D = 128, 512, 256
T = 64
G = L // T          # 8
NCORES = 8
BPC = B // NCORES   # 16 batches per core
NB = L // 128       # 4 n-blocks
F32 = mybir.dt.float32
F32R = mybir.dt.float32r
BF16 = mybir.dt.bfloat16
I32 = mybir.dt.int32
AF = mybir.ActivationFunctionType
OP = mybir.AluOpType


def bimodal_body(ctx: ExitStack, tc: "tile.TileContext", ins: dict, outs: dict):
    nc = tc.nc
    aco, vis, bag = ins["aco"], ins["vis"], ins["bag"]
    wh, bh, ww, bw, cw, cb = (
        ins["wh"], ins["bh"], ins["ww"], ins["bw"], ins["cw"], ins["cb"])
    aco_o, vis_o = outs["aco_o"], outs["vis_o"]

    const = ctx.enter_context(tc.tile_pool(name="const", bufs=1))

    # --- constant selector matrices ------------------------------------
    # Input layout: l = 4p + n  (per-partition contiguous 4KB DMA chunks),
    # so pooled frame t = l//8 = p//2 for EVERY n-block: one selector works
    # for all four n-blocks.
    # psel[p, t] = 1/8 where t == p//2, i.e. 0 <= p - 2t <= 1.
    pv = const.tile([128, 64], F32)
    nc.gpsimd.iota(pv[:], [[-2, 64]], base=0, channel_multiplier=1,
                   allow_small_or_imprecise_dtypes=True)
    ptmp = const.tile([128, 64], F32)
    nc.vector.tensor_scalar(ptmp[:], pv[:], 0.0, 1.0 / G, op0=OP.is_ge, op1=OP.mult)
    psel = const.tile([128, 64], F32)
    nc.vector.scalar_tensor_tensor(psel[:], pv[:], 1.0, ptmp[:],
                                   op0=OP.is_le, op1=OP.mult)
    # fsel[k, p]: rows 0:64 = indicator(k == p//2) (1.0), row 64 = 1.0
    # (adds the w_att row of the moving operand).  The /3 is applied exactly
    # in fp32 by the mask pass (scale = mask/3).
    fv = const.tile([64, 128], F32)
    nc.gpsimd.iota(fv[:], [[1, 128]], base=0, channel_multiplier=-2,
                   allow_small_or_imprecise_dtypes=True)
    ftmp = const.tile([64, 128], F32)
    nc.vector.tensor_scalar(ftmp[:], fv[:], 0.0, 1.0, op0=OP.is_ge, op1=OP.mult)
    ftmp2 = const.tile([65, 128], F32)
    nc.vector.scalar_tensor_tensor(ftmp2[0:64, :], fv[:], 1.0, ftmp[:],
                                   op0=OP.is_le, op1=OP.mult)
    nc.gpsimd.memset(ftmp2[64:65, :], 1.0)
    fsel = const.tile([65, 128], F32R)
    nc.vector.tensor_copy(fsel[:], ftmp2[:])

    ones_row64 = const.tile([1, 64], F32)
    nc.gpsimd.memset(ones_row64[:], 1.0)
    ones_col64 = const.tile([64, 1], F32)
    nc.gpsimd.memset(ones_col64[:], 1.0)
    ones_col64b = const.tile([64, 1], BF16)
    nc.vector.tensor_copy(ones_col64b[:], ones_col64[:])
    ones_1b = const.tile([1, 1], BF16)
    nc.vector.tensor_copy(ones_1b[:], ones_col64[0:1, :])

    # identity (for PE transposes): I[p, f] = (f == p)
    iota_p = const.tile([128, 1], F32)
    nc.gpsimd.iota(iota_p[:], [[1, 1]], base=0, channel_multiplier=1,
                   allow_small_or_imprecise_dtypes=True)
    iota_f = const.tile([128, 128], F32)
    nc.gpsimd.iota(iota_f[:], [[1, 128]], base=0, channel_multiplier=0,
                   allow_small_or_imprecise_dtypes=True)
    ident = const.tile([128, 128], F32)
    nc.vector.tensor_scalar(ident[:], iota_f[:], iota_p[:], None, op0=OP.is_equal)

    # --- parameters -----------------------------------------------------
    wh_n = const.tile([64, 64], F32)
    nc.sync.dma_start(wh_n[:], wh)
    ww_n0 = const.tile([128, 256], F32)
    nc.sync.dma_start(ww_n0[:], ww[0:128, :])
    ww_n1 = const.tile([128, 256], F32)
    nc.sync.dma_start(ww_n1[:], ww[128:256, :])
    bh_sb = const.tile([64, 1], F32)
    nc.sync.dma_start(bh_sb[:], bh[:, None])
    bw_row = const.tile([1, 256], F32)
    nc.sync.dma_start(bw_row[:], bw[None, :])
    cwb = const.tile([1, 3], F32)
    nc.sync.dma_start(cwb[:, 0:2], cw[None, :])
    nc.sync.dma_start(cwb[:, 2:3], cb[None, :])

    # bw pre-scaled so the K=1 accumulate matmul lands exactly on +bw
    # after the final sigmoid scale of 0.5/64 (128 * 0.5/64 = 1).
    bw128 = const.tile([1, 256], BF16)
    nc.vector.tensor_scalar(bw128[:], bw_row[:], 128.0, None, op0=OP.mult)

    # IS_BAG -> per-(n,b) mask columns.  Load [16b, 512l], compare ==1,
    # PE-transpose each 128-l block to get mask[p, n*16+b].
    bag_i = const.tile([16, 512], I32)
    nc.sync.dma_start(bag_i[:], bag)
    bag_f = const.tile([16, 512], F32)
    nc.vector.tensor_copy(bag_f[:], bag_i[:])
    mask_r = const.tile([16, 512], F32)
    nc.vector.tensor_scalar(mask_r[:], bag_f[:], 1.0, None, op0=OP.is_equal)

    mask_sb = const.tile([128, 64], F32)   # [p, n*16+b]
    mask3_sb = const.tile([128, 64], F32)  # mask / 3
    omb_sb = const.tile([128, 64], F32)    # 1 - mask
    whT = const.tile([64, 64], F32)        # Wh^T  [k, t]
    wwT = const.tile([128, 512], BF16)     # Ww^T  [k_local, c*256 + d]
    cvec = const.tile([64, 4], F32)        # conv scalars bcast: cw0, cw1, cb, cw0-cw1

    with tc.tile_pool(name="tpsum", bufs=2, space="PSUM") as tp:
        t_wh = tp.tile([64, 64], F32, tag="t128")
        nc.tensor.transpose(t_wh[:], wh_n[:], ident[0:64, 0:64])
        nc.scalar.copy(whT[:], t_wh[:])
        for c in range(2):
            for dh in range(2):
                src = ww_n0 if dh == 0 else ww_n1
                t_ww = tp.tile([128, 128], F32, tag="t128")
                nc.tensor.transpose(t_ww[:], src[:, c * 128:(c + 1) * 128], ident[:])
                nc.vector.tensor_copy(
                    wwT[:, c * 256 + dh * 128:c * 256 + dh * 128 + 128], t_ww[:])
        mask_v = mask_r[:].rearrange("b (p n) -> b n p", n=NB)
        for n in range(NB):
            t_bag = tp.tile([128, 16], F32, tag="t128")
            nc.tensor.transpose(t_bag[:], mask_v[:, n, :], ident[0:16, 0:16])
            nc.scalar.copy(mask_sb[:, n * 16:(n + 1) * 16], t_bag[:])
        # broadcast conv scalars across 64 partitions via K=1 matmul
        t_cv = tp.tile([64, 3], F32, tag="t128")
        nc.tensor.matmul(t_cv[:], ones_row64[:], cwb[:], start=True, stop=True)
        nc.scalar.copy(cvec[:, 0:3], t_cv[:])

    nc.vector.tensor_scalar(mask3_sb[:], mask_sb[:], 1.0 / 3.0, None, op0=OP.mult)
    nc.vector.tensor_scalar(omb_sb[:], mask_sb[:], -1.0, 1.0, op0=OP.mult, op1=OP.add)
    nc.vector.tensor_tensor(cvec[:, 3:4], cvec[:, 0:1], cvec[:, 1:2], op=OP.subtract)

    # --- per-batch pools ------------------------------------------------
    io_in = ctx.enter_context(tc.tile_pool(name="io_in", bufs=4))
    io_out = ctx.enter_context(tc.tile_pool(name="io_out", bufs=3))
    scale_pool = ctx.enter_context(tc.tile_pool(name="scale", bufs=3))
    small = ctx.enter_context(tc.tile_pool(name="small", bufs=3))
    pp = ctx.enter_context(tc.tile_pool(name="pp", bufs=1, space="PSUM"))
    pst = ctx.enter_context(tc.tile_pool(name="pst", bufs=2, space="PSUM"))
    pfull = ctx.enter_context(tc.tile_pool(name="pfull", bufs=2, space="PSUM"))

    # PE_HAM warm-up: ~14 dense back-to-back matmuls (~6 us) flip the PE
    # clock gate to 8/8 (2.4 GHz); the main loop's duty cycle then keeps it
    # there (de-warm needs a fully idle ~3.4 us window).
    prime = pst.tile([128, 512], F32, tag="st")
    for _ in range(14):
        nc.tensor.matmul(prime[:, 0:128], ident[:], ident[:],
                         start=True, stop=True)

    for b in range(BPC):
        # ---- load [512, 256] as [128, (n, d)], l = n*128 + p ----
        a_in = io_in.tile([128, NB * 256], F32, tag="a_in")
        nc.sync.dma_start(
            a_in[:].rearrange("p (n d) -> p n d", d=256),
            aco[b].rearrange("(p n) d -> p n d", n=NB))
        v_in = io_in.tile([128, NB * 256], F32, tag="v_in")
        nc.sync.dma_start(
            v_in[:].rearrange("p (n d) -> p n d", d=256),
            vis[b].rearrange("(p n) d -> p n d", n=NB))

        # ---- pooled sums: pa = aco_p, pv = vis_p (fp32 matmuls; no input
        # casts needed, and a cold 8-matmul fp32 burst is long enough to
        # re-warm the PE clock gate by itself) ----
        pa_t = pp.tile([64, 256], F32, tag="pa")
        pv_t = pp.tile([64, 256], F32, tag="pv")
        for n in range(NB):
            blk = slice(n * 256, (n + 1) * 256)
            nc.tensor.matmul(pa_t[:], psel[:], a_in[:, blk],
                             start=(n == 0), stop=(n == NB - 1))
        for n in range(NB):
            blk = slice(n * 256, (n + 1) * 256)
            nc.tensor.matmul(pv_t[:], psel[:], v_in[:, blk],
                             start=(n == 0), stop=(n == NB - 1))

        # ---- c_pre = cw0*aco_p + cw1*vis_p + cb ----
        s1 = small.tile([64, 256], F32, tag="s1")
        nc.scalar.activation(s1[:], pv_t[:], AF.Identity,
                             bias=cvec[:, 2:3], scale=cvec[:, 1:2])
        c_pre = small.tile([64, 256], F32, tag="c_pre")
        nc.vector.scalar_tensor_tensor(c_pre[:], pa_t[:], cvec[:, 0:1], s1[:],
                                       op0=OP.mult, op1=OP.add)
        c_att = small.tile([64, 256], F32, tag="c_att")
        nc.scalar.activation(c_att[:], c_pre[:], AF.Sigmoid)

        # ---- hw = aco_p + vis_p in SBUF (bf16) + row sums in one op ----
        acop = small.tile([64, 256], F32, tag="acop")
        nc.scalar.copy(acop[:], pa_t[:])
        hw_sb = small.tile([64, 256], BF16, tag="hw_sb")
        hmean = small.tile([64, 1], F32, tag="hmean")
        nc.vector.scalar_tensor_tensor(hw_sb[:], pv_t[:], 0.0, acop[:],
                                       op0=OP.add, op1=OP.add,
                                       accum_out=hmean[:])

        # ---- stats matmuls ----
        st = pst.tile([128, 512], F32, tag="st")
        # wsum[d] = sum_t hw[t, d]  (two d-halves -> columns 0,1)
        nc.tensor.matmul(st[:, 0:1], hw_sb[:, 0:128], ones_col64b[:],
                         start=True, stop=True)
        nc.tensor.matmul(st[:, 1:2], hw_sb[:, 128:256], ones_col64b[:],
                         start=True, stop=True)
        # h_pre[t] = sum_k Wh[t,k] * hmean_raw[k]   -> column 2
        nc.tensor.matmul(st[0:64, 2:3], whT[:], hmean[:], start=True, stop=True)
        h_att = small.tile([64, 1], F32, tag="h_att")
        nc.scalar.activation(h_att[:], st[0:64, 2:3], AF.Sigmoid,
                             bias=bh_sb[:], scale=0.5 / 256.0)

        wsum = small.tile([128, 2], BF16, tag="wsum")
        nc.scalar.copy(wsum[:], st[:, 0:2])
        # w_row[d] = sum_k Ww[d,k]*wsum[k] + 128*bw[d]   (psum row, cols 256:512)
        nc.tensor.matmul(st[0:1, 256:512], wsum[:, 0:1], wwT[:, 0:256],
                         start=True, stop=False)
        nc.tensor.matmul(st[0:1, 256:512], wsum[:, 1:2], wwT[:, 256:512],
                         start=False, stop=False)
        nc.tensor.matmul(st[0:1, 256:512], ones_1b[:], bw128[:],
                         start=False, stop=True)

        # ---- moving operand for the broadcast: rows 0:64 = c_att + h_att,
        #      row 64 = w_att ----
        scale_in = small.tile([65, 256], F32R, tag="scale_in")
        nc.scalar.activation(scale_in[0:64, :], c_att[:], AF.Identity,
                             bias=h_att[:], scale=1.0)
        nc.scalar.activation(scale_in[64:65, :], st[0:1, 256:512], AF.Sigmoid,
                             scale=0.5 / 64.0)

        # ---- broadcast pooled scale sum to full L (f32r, K=65) ----
        # moving operand re-streams scale_in twice per matmul via a
        # stride-0 AP dim; one matmul per psum bank (512 f32).
        full_t = pfull.tile([128, NB * 256], F32)
        si = scale_in[:]
        mov = bass.AP(si.tensor, si.offset, [si.ap[0], [0, 2], [1, 256]])
        for h in range(2):
            nc.tensor.matmul(
                full_t[:, h * 512:(h + 1) * 512].rearrange(
                    "p (n d) -> p n d", d=256),
                fsel[:], mov, start=True, stop=True)

        # ---- mask + /3:  scale_eff = (mask/3) * sum + (1 - mask) ----
        scale_sb = scale_pool.tile([128, NB * 256], F32, tag="scale")
        for n in range(NB):
            col = n * 16 + b
            nc.scalar.activation(scale_sb[:, n * 256:(n + 1) * 256],
                                 full_t[:, n * 256:(n + 1) * 256], AF.Identity,
                                 bias=omb_sb[:, col:col + 1],
                                 scale=mask3_sb[:, col:col + 1])

        # ---- apply + store ----
        a_out = io_out.tile([128, NB * 256], F32, tag="a_out")
        nc.vector.tensor_tensor(a_out[:], a_in[:], scale_sb[:], op=OP.mult)
        v_out = io_out.tile([128, NB * 256], F32, tag="v_out")
        nc.vector.tensor_tensor(v_out[:], v_in[:], scale_sb[:], op=OP.mult)

        nc.sync.dma_start(
            aco_o[b].rearrange("(p n) d -> p n d", n=NB),
            a_out[:].rearrange("p (n d) -> p n d", d=256))
        nc.sync.dma_start(
            vis_o[b].rearrange("(p n) d -> p n d", n=NB),
            v_out[:].rearrange("p (n d) -> p n d", d=256))


def build_nc():
    nc = bacc.Bacc("TRN2", target_bir_lowering=False, debug=False,
                   num_devices=NCORES)
    ins = {
        "aco": nc.dram_tensor("aco", [BPC, L, D], F32, kind="ExternalInput").ap(),
        "vis": nc.dram_tensor("vis", [BPC, L, D], F32, kind="ExternalInput").ap(),
        "bag": nc.dram_tensor("bag", [BPC, L], I32, kind="ExternalInput").ap(),
        "wh": nc.dram_tensor("wh", [T, T], F32, kind="ExternalInput").ap(),
        "bh": nc.dram_tensor("bh", [T], F32, kind="ExternalInput").ap(),
        "ww": nc.dram_tensor("ww", [D, D], F32, kind="ExternalInput").ap(),
        "bw": nc.dram_tensor("bw", [D], F32, kind="ExternalInput").ap(),
        "cw": nc.dram_tensor("cw", [2], F32, kind="ExternalInput").ap(),
        "cb": nc.dram_tensor("cb", [1], F32, kind="ExternalInput").ap(),
    }
    outs = {
        "aco_o": nc.dram_tensor("aco_o", [BPC, L, D], F32, kind="ExternalOutput").ap(),
        "vis_o": nc.dram_tensor("vis_o", [BPC, L, D], F32, kind="ExternalOutput").ap(),
    }
    with tile.TileContext(nc) as tc:
        with ExitStack() as ctx:
            bimodal_body(ctx, tc, ins, outs)
    nc.compile()
    return nc


_NC_CACHE = None


def _get_nc():
    global _NC_CACHE
    if _NC_CACHE is None:
        _NC_CACHE = build_nc()
    return _NC_CACHE


def _run(inputs: dict, trace: bool = False, tmpdir=None):
    nc = _get_nc()
    acoustic = np.ascontiguousarray(np.asarray(inputs["acoustic_seq"], dtype=np.float32))
    visual = np.ascontiguousarray(np.asarray(inputs["visual_seq"], dtype=np.float32))
    isbag = np.ascontiguousarray(np.asarray(inputs["IS_BAG_list"], dtype=np.int32))
    shared = {
        "wh": np.ascontiguousarray(np.asarray(inputs["Wh"], dtype=np.float32)),
        "bh": np.ascontiguousarray(np.asarray(inputs["bh"], dtype=np.float32)),
        "ww": np.ascontiguousarray(np.asarray(inputs["Ww"], dtype=np.float32)),
        "bw": np.ascontiguousarray(np.asarray(inputs["bw"], dtype=np.float32)),
        "cw": np.ascontiguousarray(np.asarray(inputs["conv_w"], dtype=np.float32)),
        "cb": np.ascontiguousarray(np.asarray(inputs["conv_b"], dtype=np.float32)),
    }
    in_maps = []
    for m in range(NCORES):
        sl = slice(m * BPC, (m + 1) * BPC)
        in_maps.append({
            "aco": acoustic[sl], "vis": visual[sl], "bag": isbag[sl], **shared})
    res = run_bass_kernel_spmd(nc, in_maps, core_ids=list(range(NCORES)),
                               trace=trace, tmpdir=tmpdir)
    aco_out = np.concatenate([res.results[m]["aco_o"] for m in range(NCORES)], axis=0)
    vis_out = np.concatenate([res.results[m]["vis_o"] for m in range(NCORES)], axis=0)
    return (aco_out, vis_out), res


def kernel(**inputs) -> np.ndarray:
    (aco_out, vis_out), _ = _run(inputs)
    return aco_out, vis_out


# revision 24
# speedup vs baseline: 1.1803x; 1.1803x over previous
"""Trainium2 Bass kernel for nn_BimodalAttention.

Reference computation (B=128, L=512, D=256, T=64, G=8):
  aco_p/vis_p = group-mean pool (8->1) along L            [B,T,D]
  c_att = sigmoid(cw0*aco_p + cw1*vis_p + cb)             [B,T,D]
  hw    = 0.5*(aco_p+vis_p)
  h_att = sigmoid(hw.mean(D) @ Wh.T + bh)                 [B,T]
  w_att = sigmoid(hw.mean(T) @ Ww.T + bw)                 [B,D]
  scale = (h_att[map] + w_att + c_att[map]) / 3           [B,L,D]
  out   = where(IS_BAG==1, in*scale, in)  for both modalities

Sharding: pure data parallel, 16 batches per core on 8 cores.

Per-core on-chip layout: batch slice [512,256] viewed as [128p, (n=4, d=256)]
with l = n*128 + p, so pooled frame t = 16n + p//8.  Pooling runs on the PE
in bf16 (exact 1/8 selector weights); the pooled->full broadcast runs in
float32r with the w_att row folded in as a 65th contraction row; the /3 and
the IS_BAG mask are applied exactly in fp32 via a per-partition
scale/bias pass (scale = mask/3, bias = 1-mask).
"""

import sys
from contextlib import ExitStack

import numpy as np

sys.path.insert(0, "/opt/trn_rl_repo")

import concourse.bass as bass  # noqa: E402
import concourse.tile as tile  # noqa: E402
from concourse import bacc, mybir  # noqa: E402
from concourse.bass_utils import run_bass_kernel_spmd  # noqa: E402

B, L, D = 128, 512, 256
T = 64
G = L // T          # 8
NCORES = 8
BPC = B // NCORES   # 16 batches per core
NB = L // 128       # 4 n-blocks
F32 = mybir.dt.float32
F32R = mybir.dt.float32r
BF16 = mybir.dt.bfloat16
I32 = mybir.dt.int32
AF = mybir.ActivationFunctionType
OP = mybir.AluOpType


def bimodal_body(ctx: ExitStack, tc: "tile.TileContext", ins: dict, outs: dict):
    nc = tc.nc
    aco, vis, bag = ins["aco"], ins["vis"], ins["bag"]
    wh, bh, ww, bw, cw, cb = (
        ins["wh"], ins["bh"], ins["ww"], ins["bw"], ins["cw"], ins["cb"])
    aco_o, vis_o = outs["aco_o"], outs["vis_o"]

    const = ctx.enter_context(tc.tile_pool(name="const", bufs=1))

    # --- constant selector matrices ------------------------------------
    # Input layout: l = 4p + n  (per-partition contiguous 4KB DMA chunks),
    # so pooled frame t = l//8 = p//2 for EVERY n-block: one selector works
    # for all four n-blocks.
    # psel[p, t] = 1/8 where t == p//2, i.e. 0 <= p - 2t <= 1.
    pv = const.tile([128, 64], F32)
    nc.gpsimd.iota(pv[:], [[-2, 64]], base=0, channel_multiplier=1,
                   allow_small_or_imprecise_dtypes=True)
    ptmp = const.tile([128, 64], F32)
    nc.vector.tensor_scalar(ptmp[:], pv[:], 0.0, 1.0 / G, op0=OP.is_ge, op1=OP.mult)
    psel = const.tile([128, 64], BF16)
    nc.vector.scalar_tensor_tensor(psel[:], pv[:], 1.0, ptmp[:],
                                   op0=OP.is_le, op1=OP.mult)
    # fsel[k, p]: rows 0:64 = indicator(k == p//2) (1.0), row 64 = 1.0
    # (adds the w_att row of the moving operand).  The /3 is applied exactly
    # in fp32 by the mask pass (scale = mask/3).
    fv = const.tile([64, 128], F32)
    nc.gpsimd.iota(fv[:], [[1, 128]], base=0, channel_multiplier=-2,
                   allow_small_or_imprecise_dtypes=True)
    ftmp = const.tile([64, 128], F32)
    nc.vector.tensor_scalar(ftmp[:], fv[:], 0.0, 1.0, op0=OP.is_ge, op1=OP.mult)
    ftmp2 = const.tile([65, 128], F32)
    nc.vector.scalar_tensor_tensor(ftmp2[0:64, :], fv[:], 1.0, ftmp[:],
                                   op0=OP.is_le, op1=OP.mult)
    nc.gpsimd.memset(ftmp2[64:65, :], 1.0)
    fsel = const.tile([65, 128], F32R)
    nc.vector.tensor_copy(fsel[:], ftmp2[:])

    ones_row64 = const.tile([1, 64], F32)
    nc.gpsimd.memset(ones_row64[:], 1.0)
    ones_col64 = const.tile([64, 1], F32)
    nc.gpsimd.memset(ones_col64[:], 1.0)
    ones_col64b = const.tile([64, 1], BF16)
    nc.vector.tensor_copy(ones_col64b[:], ones_col64[:])
    ones_1b = const.tile([1, 1], BF16)
    nc.vector.tensor_copy(ones_1b[:], ones_col64[0:1, :])

    # identity (for PE transposes): I[p, f] = (f == p)
    iota_p = const.tile([128, 1], F32)
    nc.gpsimd.iota(iota_p[:], [[1, 1]], base=0, channel_multiplier=1,
                   allow_small_or_imprecise_dtypes=True)
    iota_f = const.tile([128, 128], F32)
    nc.gpsimd.iota(iota_f[:], [[1, 128]], base=0, channel_multiplier=0,
                   allow_small_or_imprecise_dtypes=True)
    ident = const.tile([128, 128], F32)
    nc.vector.tensor_scalar(ident[:], iota_f[:], iota_p[:], None, op0=OP.is_equal)

    # --- parameters -----------------------------------------------------
    wh_n = const.tile([64, 64], F32)
    nc.sync.dma_start(wh_n[:], wh)
    ww_n0 = const.tile([128, 256], F32)
    nc.sync.dma_start(ww_n0[:], ww[0:128, :])
    ww_n1 = const.tile([128, 256], F32)
    nc.sync.dma_start(ww_n1[:], ww[128:256, :])
    bh_sb = const.tile([64, 1], F32)
    nc.sync.dma_start(bh_sb[:], bh[:, None])
    bw_row = const.tile([1, 256], F32)
    nc.sync.dma_start(bw_row[:], bw[None, :])
    cwb = const.tile([1, 3], F32)
    nc.sync.dma_start(cwb[:, 0:2], cw[None, :])
    nc.sync.dma_start(cwb[:, 2:3], cb[None, :])

    # bw pre-scaled so the K=1 accumulate matmul lands exactly on +bw
    # after the final sigmoid scale of 0.5/64 (128 * 0.5/64 = 1).
    bw128 = const.tile([1, 256], BF16)
    nc.vector.tensor_scalar(bw128[:], bw_row[:], 128.0, None, op0=OP.mult)

    # IS_BAG -> per-(n,b) mask columns.  Load [16b, 512l], compare ==1,
    # PE-transpose each 128-l block to get mask[p, n*16+b].
    bag_i = const.tile([16, 512], I32)
    nc.sync.dma_start(bag_i[:], bag)
    bag_f = const.tile([16, 512], F32)
    nc.vector.tensor_copy(bag_f[:], bag_i[:])
    mask_r = const.tile([16, 512], F32)
    nc.vector.tensor_scalar(mask_r[:], bag_f[:], 1.0, None, op0=OP.is_equal)

    mask_sb = const.tile([128, 64], F32)   # [p, n*16+b]
    mask3_sb = const.tile([128, 64], F32)  # mask / 3
    omb_sb = const.tile([128, 64], F32)    # 1 - mask
    whT = const.tile([64, 64], F32)        # Wh^T  [k, t]
    wwT = const.tile([128, 512], BF16)     # Ww^T  [k_local, c*256 + d]
    cvec = const.tile([64, 4], F32)        # conv scalars bcast: cw0, cw1, cb, cw0-cw1

    with tc.tile_pool(name="tpsum", bufs=2, space="PSUM") as tp:
        t_wh = tp.tile([64, 64], F32, tag="t128")
        nc.tensor.transpose(t_wh[:], wh_n[:], ident[0:64, 0:64])
        nc.scalar.copy(whT[:], t_wh[:])
        for c in range(2):
            for dh in range(2):
                src = ww_n0 if dh == 0 else ww_n1
                t_ww = tp.tile([128, 128], F32, tag="t128")
                nc.tensor.transpose(t_ww[:], src[:, c * 128:(c + 1) * 128], ident[:])
                nc.vector.tensor_copy(
                    wwT[:, c * 256 + dh * 128:c * 256 + dh * 128 + 128], t_ww[:])
        mask_v = mask_r[:].rearrange("b (p n) -> b n p", n=NB)
        for n in range(NB):
            t_bag = tp.tile([128, 16], F32, tag="t128")
            nc.tensor.transpose(t_bag[:], mask_v[:, n, :], ident[0:16, 0:16])
            nc.scalar.copy(mask_sb[:, n * 16:(n + 1) * 16], t_bag[:])
        # broadcast conv scalars across 64 partitions via K=1 matmul
        t_cv = tp.tile([64, 3], F32, tag="t128")
        nc.tensor.matmul(t_cv[:], ones_row64[:], cwb[:], start=True, stop=True)
        nc.scalar.copy(cvec[:, 0:3], t_cv[:])

    nc.vector.tensor_scalar(mask3_sb[:], mask_sb[:], 1.0 / 3.0, None, op0=OP.mult)
    nc.vector.tensor_scalar(omb_sb[:], mask_sb[:], -1.0, 1.0, op0=OP.mult, op1=OP.add)
    nc.vector.tensor_tensor(cvec[:, 3:4], cvec[:, 0:1], cvec[:, 1:2], op=OP.subtract)

    # --- per-batch pools ------------------------------------------------
    io_in = ctx.enter_context(tc.tile_pool(name="io_in", bufs=5))
    io_out = ctx.enter_context(tc.tile_pool(name="io_out", bufs=3))
    scale_pool = ctx.enter_context(tc.tile_pool(name="scale", bufs=3))
    small = ctx.enter_context(tc.tile_pool(name="small", bufs=3))
    pp = ctx.enter_context(tc.tile_pool(name="pp", bufs=1, space="PSUM"))
    pst = ctx.enter_context(tc.tile_pool(name="pst", bufs=2, space="PSUM"))
    pfull = ctx.enter_context(tc.tile_pool(name="pfull", bufs=2, space="PSUM"))

    # PE_HAM warm-up: ~14 dense back-to-back matmuls (~6 us) flip the PE
    # clock gate to 8/8 (2.4 GHz); the main loop's duty cycle then keeps it
    # there (de-warm needs a fully idle ~3.4 us window).
    prime = pst.tile([128, 512], F32, tag="st")
    for _ in range(14):
        nc.tensor.matmul(prime[:, 0:128], ident[:], ident[:],
                         start=True, stop=True)

    for b in range(BPC):
        # ---- load [512, 256] as [128, (n, d)], l = n*128 + p ----
        a_in = io_in.tile([128, NB * 256], F32, tag="a_in")
        nc.sync.dma_start(
            a_in[:].rearrange("p (n d) -> p n d", d=256),
            aco[b].rearrange("(p n) d -> p n d", n=NB))
        v_in = io_in.tile([128, NB * 256], F32, tag="v_in")
        nc.sync.dma_start(
            v_in[:].rearrange("p (n d) -> p n d", d=256),
            vis[b].rearrange("(p n) d -> p n d", n=NB))

        # bf16 copies for the pooling matmuls (DVE ~600ns each via 2x mode)
        a_r = io_in.tile([128, NB * 256], BF16, tag="a_r")
        nc.vector.tensor_copy(a_r[:], a_in[:])
        v_r = io_in.tile([128, NB * 256], BF16, tag="v_r")
        nc.vector.tensor_copy(v_r[:], v_in[:])

        # ---- pooled sums: pa = aco_p, pv = vis_p (bf16 matmuls) ----
        pa_t = pp.tile([64, 256], F32, tag="pa")
        pv_t = pp.tile([64, 256], F32, tag="pv")
        for n in range(NB):
            blk = slice(n * 256, (n + 1) * 256)
            nc.tensor.matmul(pa_t[:], psel[:], a_r[:, blk],
                             start=(n == 0), stop=(n == NB - 1))
        for n in range(NB):
            blk = slice(n * 256, (n + 1) * 256)
            nc.tensor.matmul(pv_t[:], psel[:], v_r[:, blk],
                             start=(n == 0), stop=(n == NB - 1))

        # ---- c_pre = cw0*aco_p + cw1*vis_p + cb ----
        s1 = small.tile([64, 256], F32, tag="s1")
        nc.scalar.activation(s1[:], pv_t[:], AF.Identity,
                             bias=cvec[:, 2:3], scale=cvec[:, 1:2])
        c_pre = small.tile([64, 256], F32, tag="c_pre")
        nc.vector.scalar_tensor_tensor(c_pre[:], pa_t[:], cvec[:, 0:1], s1[:],
                                       op0=OP.mult, op1=OP.add)
        c_att = small.tile([64, 256], F32, tag="c_att")
        nc.scalar.activation(c_att[:], c_pre[:], AF.Sigmoid)

        # ---- hw = aco_p + vis_p in SBUF (bf16) + row sums in one op ----
        acop = small.tile([64, 256], F32, tag="acop")
        nc.scalar.copy(acop[:], pa_t[:])
        hw_sb = small.tile([64, 256], BF16, tag="hw_sb")
        hmean = small.tile([64, 1], F32, tag="hmean")
        nc.vector.scalar_tensor_tensor(hw_sb[:], pv_t[:], 0.0, acop[:],
                                       op0=OP.add, op1=OP.add,
                                       accum_out=hmean[:])

        # ---- stats matmuls ----
        st = pst.tile([128, 512], F32, tag="st")
        # wsum[d] = sum_t hw[t, d]  (two d-halves -> columns 0,1)
        nc.tensor.matmul(st[:, 0:1], hw_sb[:, 0:128], ones_col64b[:],
                         start=True, stop=True)
        nc.tensor.matmul(st[:, 1:2], hw_sb[:, 128:256], ones_col64b[:],
                         start=True, stop=True)
        # h_pre[t] = sum_k Wh[t,k] * hmean_raw[k]   -> column 2
        nc.tensor.matmul(st[0:64, 2:3], whT[:], hmean[:], start=True, stop=True)
        h_att = small.tile([64, 1], F32, tag="h_att")
        nc.scalar.activation(h_att[:], st[0:64, 2:3], AF.Sigmoid,
                             bias=bh_sb[:], scale=0.5 / 256.0)

        wsum = small.tile([128, 2], BF16, tag="wsum")
        nc.scalar.copy(wsum[:], st[:, 0:2])
        # w_row[d] = sum_k Ww[d,k]*wsum[k] + 128*bw[d]   (psum row, cols 256:512)
        nc.tensor.matmul(st[0:1, 256:512], wsum[:, 0:1], wwT[:, 0:256],
                         start=True, stop=False)
        nc.tensor.matmul(st[0:1, 256:512], wsum[:, 1:2], wwT[:, 256:512],
                         start=False, stop=False)
        nc.tensor.matmul(st[0:1, 256:512], ones_1b[:], bw128[:],
                         start=False, stop=True)

        # ---- moving operand for the broadcast: rows 0:64 = c_att + h_att,
        #      row 64 = w_att ----
        scale_in = small.tile([65, 256], F32R, tag="scale_in")
        nc.scalar.activation(scale_in[0:64, :], c_att[:], AF.Identity,
                             bias=h_att[:], scale=1.0)
        nc.scalar.activation(scale_in[64:65, :], st[0:1, 256:512], AF.Sigmoid,
                             scale=0.5 / 64.0)

        # ---- broadcast pooled scale sum to full L (f32r, K=65) ----
        # moving operand re-streams scale_in twice per matmul via a
        # stride-0 AP dim; one matmul per psum bank (512 f32).
        full_t = pfull.tile([128, NB * 256], F32)
        si = scale_in[:]
        mov = bass.AP(si.tensor, si.offset, [si.ap[0], [0, 2], [1, 256]])
        for h in range(2):
            nc.tensor.matmul(
                full_t[:, h * 512:(h + 1) * 512].rearrange(
                    "p (n d) -> p n d", d=256),
                fsel[:], mov, start=True, stop=True)

        # ---- mask + /3:  scale_eff = (mask/3) * sum + (1 - mask) ----
        scale_sb = scale_pool.tile([128, NB * 256], F32, tag="scale")
        for n in range(NB):
            col = n * 16 + b
            nc.scalar.activation(scale_sb[:, n * 256:(n + 1) * 256],
                                 full_t[:, n * 256:(n + 1) * 256], AF.Identity,
                                 bias=omb_sb[:, col:col + 1],
                                 scale=mask3_sb[:, col:col + 1])

        # ---- apply + store ----
        a_out = io_out.tile([128, NB * 256], F32, tag="a_out")
        nc.vector.tensor_tensor(a_out[:], a_in[:], scale_sb[:], op=OP.mult)
        v_out = io_out.tile([128, NB * 256], F32, tag="v_out")
        nc.vector.tensor_tensor(v_out[:], v_in[:], scale_sb[:], op=OP.mult)

        nc.sync.dma_start(
            aco_o[b].rearrange("(p n) d -> p n d", n=NB),
            a_out[:].rearrange("p (n d) -> p n d", d=256))
        nc.sync.dma_start(
            vis_o[b].rearrange("(p n) d -> p n d", n=NB),
            v_out[:].rearrange("p (n d) -> p n d", d=256))


def build_nc():
    nc = bacc.Bacc("TRN2", target_bir_lowering=False, debug=False,
                   num_devices=NCORES)
    ins = {
        "aco": nc.dram_tensor("aco", [BPC, L, D], F32, kind="ExternalInput").ap(),
        "vis": nc.dram_tensor("vis", [BPC, L, D], F32, kind="ExternalInput").ap(),
        "bag": nc.dram_tensor("bag", [BPC, L], I32, kind="ExternalInput").ap(),
        "wh": nc.dram_tensor("wh", [T, T], F32, kind="ExternalInput").ap(),
        "bh": nc.dram_tensor("bh", [T], F32, kind="ExternalInput").ap(),
        "ww": nc.dram_tensor("ww", [D, D], F32, kind="ExternalInput").ap(),
        "bw": nc.dram_tensor("bw", [D], F32, kind="ExternalInput").ap(),
        "cw": nc.dram_tensor("cw", [2], F32, kind="ExternalInput").ap(),
        "cb": nc.dram_tensor("cb", [1], F32, kind="ExternalInput").ap(),
    }
    outs = {
        "aco_o": nc.dram_tensor("aco_o", [BPC, L, D], F32, kind="ExternalOutput").ap(),
        "vis_o": nc.dram_tensor("vis_o", [BPC, L, D], F32, kind="ExternalOutput").ap(),
    }
    with tile.TileContext(nc) as tc:
        with ExitStack() as ctx:
            bimodal_body(ctx, tc, ins, outs)
    nc.compile()
    return nc


_NC_CACHE = None


def _get_nc():
    global _NC_CACHE
    if _NC_CACHE is None:
        _NC_CACHE = build_nc()
    return _NC_CACHE


def _run(inputs: dict, trace: bool = False, tmpdir=None):
    nc = _get_nc()
    acoustic = np.ascontiguousarray(np.asarray(inputs["acoustic_seq"], dtype=np.float32))
    visual = np.ascontiguousarray(np.asarray(inputs["visual_seq"], dtype=np.float32))
    isbag = np.ascontiguousarray(np.asarray(inputs["IS_BAG_list"], dtype=np.int32))
    shared = {
        "wh": np.ascontiguousarray(np.asarray(inputs["Wh"], dtype=np.float32)),
        "bh": np.ascontiguousarray(np.asarray(inputs["bh"], dtype=np.float32)),
        "ww": np.ascontiguousarray(np.asarray(inputs["Ww"], dtype=np.float32)),
        "bw": np.ascontiguousarray(np.asarray(inputs["bw"], dtype=np.float32)),
        "cw": np.ascontiguousarray(np.asarray(inputs["conv_w"], dtype=np.float32)),
        "cb": np.ascontiguousarray(np.asarray(inputs["conv_b"], dtype=np.float32)),
    }
    in_maps = []
    for m in range(NCORES):
        sl = slice(m * BPC, (m + 1) * BPC)
        in_maps.append({
            "aco": acoustic[sl], "vis": visual[sl], "bag": isbag[sl], **shared})
    res = run_bass_kernel_spmd(nc, in_maps, core_ids=list(range(NCORES)),
                               trace=trace, tmpdir=tmpdir)
    aco_out = np.concatenate([res.results[m]["aco_o"] for m in range(NCORES)], axis=0)
    vis_out = np.concatenate([res.results[m]["vis_o"] for m in range(NCORES)], axis=0)
    return (aco_out, vis_out), res


def kernel(**inputs) -> np.ndarray:
    (aco_out, vis_out), _ = _run(inputs)
    return aco_out, vis_out
